# revision 2
# baseline (speedup 1.0000x reference)
"""Trainium2 Bass kernel for nn_LIMADNN2_42013370090068 (dense_mlp).

Reference semantics: out depends only on x[:, 0, :] — the `state.add(...)`
neighbor loop in the torch module is not in-place, so the 65-neighbor
dimension is dead. force_prev = x[:, 0, 6:9] is a pure slice.

  q   = x[:, 0, :]                 # [B, 12]
  h   = relu(q @ W1 + b1)          # [B, 16]
  blk = relu(h @ W2 + b2)          # [B, 8]
  out = (blk @ Ws + bs) @ Wo + bo  # [B, 3]   (no relu between -> folded)

Device strategy (pure data parallel, 8 cores, batch-sharded):
  * Host slices q (12.6 MB of the 818 MB input), computes force_prev, and
    folds Ws/Wo into one [8,3] matrix (no nonlinearity between them).
  * Features-on-partitions layout: matmuls stream atoms along the PSUM
    free dimension (N=512) with lhsT = weights.
  * 8 batch-chunks packed per PE pass via block-diagonal weights:
    W1_bd [96,128], W2_bd [128,64], W3_bd [64,24]. One matmul therefore
    processes 8x512 = 4096 atoms.
  * DMA count minimized (descriptor-gen serializes): all weights+biases
    ride one packed [128, 218] DMA; inputs in 4 DMAs, outputs in 2.
  * Activations work on 1024-wide pairs of matmul outputs to amortize
    fixed per-op cost; biases fused (ScalarE relu, VectorE dual-op
    tensor_scalar). Final bias bso added on host.
"""

import numpy as np

B = 262144
F = 12
N_CORES = 8
BPC = B // N_CORES          # 32768 atoms per core
CHUNKS = 8                  # batch chunks packed on PE partitions
TILE_N = 512                # atoms per matmul column tile (fp32 PSUM bank)
SUPER = BPC // (CHUNKS * TILE_N)   # 8 supertiles per core
FREE = SUPER * TILE_N       # 4096
WCOLS = 218                 # packed weight tensor columns

# matmul operand dtype: "float32" (exact, 4 cyc/row) or "float32r"
# (1 cyc/row at N=512; reduced-precision fp32 mode)
MM_DTYPE = "float32"


def _build_nc():
    import concourse.tile as tile
    from concourse import bacc, mybir

    f32 = mybir.dt.float32
    mmdt = getattr(mybir.dt, MM_DTYPE)

    nc = bacc.Bacc("TRN2", target_bir_lowering=False, debug=False,
                   num_devices=N_CORES)

    xin = nc.dram_tensor("xin", [CHUNKS * F, FREE], f32, kind="ExternalInput")
    wpack = nc.dram_tensor("wpack", [128, WCOLS], f32, kind="ExternalInput")
    out = nc.dram_tensor("out", [24, FREE], f32, kind="ExternalOutput")

    Relu = mybir.ActivationFunctionType.Relu
    add, vmax = mybir.AluOpType.add, mybir.AluOpType.max

    def mm(ps_ap, lhsT_ap, rhs_ap):
        nc.tensor.matmul(ps_ap, lhsT_ap.bitcast(mmdt), rhs_ap.bitcast(mmdt),
                         start=True, stop=True)

    with tile.TileContext(nc) as tc:
        with (
            tc.tile_pool(name="const", bufs=1) as cpool,
            tc.tile_pool(name="xt", bufs=2) as xpool,
            tc.tile_pool(name="h", bufs=2) as hpool,
            tc.tile_pool(name="blk", bufs=2) as bpool,
            tc.tile_pool(name="osb", bufs=2) as opool,
            tc.tile_pool(name="ps1", bufs=2, space="PSUM") as ps1pool,
            tc.tile_pool(name="ps2", bufs=1, space="PSUM") as ps2pool,
            tc.tile_pool(name="ps3", bufs=1, space="PSUM") as ps3pool,
        ):
            wsb = cpool.tile([128, WCOLS], f32)
            nc.sync.dma_start(wsb[:], wpack[:])
            w1_ap = wsb[0:96, 0:128]
            w2_ap = wsb[0:128, 128:192]
            w3_ap = wsb[0:64, 192:216]
            b1_ap = wsb[0:128, 216:217]
            b2_ap = wsb[0:64, 217:218]

            osb = None
            for p in range(SUPER // 2):
                cols = slice(2 * p * TILE_N, 2 * (p + 1) * TILE_N)
                xt = xpool.tile([96, 2 * TILE_N], f32)
                nc.sync.dma_start(xt[:], xin[:, cols])

                ps1 = ps1pool.tile([128, 2 * TILE_N], f32)
                mm(ps1[:, 0:TILE_N], w1_ap, xt[:, 0:TILE_N])
                mm(ps1[:, TILE_N:], w1_ap, xt[:, TILE_N:])
                h = hpool.tile([128, 2 * TILE_N], f32)
                nc.scalar.activation(h[:], ps1[:], Relu, bias=b1_ap)

                ps2 = ps2pool.tile([64, 2 * TILE_N], f32)
                mm(ps2[:, 0:TILE_N], w2_ap, h[:, 0:TILE_N])
                mm(ps2[:, TILE_N:], w2_ap, h[:, TILE_N:])
                blk = bpool.tile([64, 2 * TILE_N], f32)
                nc.vector.tensor_scalar(blk[:], ps2[:], b2_ap, 0.0, add, vmax)

                ps3 = ps3pool.tile([24, 2 * TILE_N], f32)
                mm(ps3[:, 0:TILE_N], w3_ap, blk[:, 0:TILE_N])
                mm(ps3[:, TILE_N:], w3_ap, blk[:, TILE_N:])

                if p % 2 == 0:
                    osb = opool.tile([24, 4 * TILE_N], f32)
                    nc.scalar.copy(osb[:, 0:2 * TILE_N], ps3[:])
                else:
                    nc.vector.tensor_copy(osb[:, 2 * TILE_N:], ps3[:])
                    nc.sync.dma_start(
                        out[:, (p - 1) * 2 * TILE_N:(p + 1) * 2 * TILE_N],
                        osb[:])

    nc.finalize()
    return nc


def _host_prep(x, W1, b1, W2, b2, Ws, bs, Wo, bo):
    x = np.asarray(x)
    W1 = np.asarray(W1, dtype=np.float32)
    b1 = np.asarray(b1, dtype=np.float32)
    W2 = np.asarray(W2, dtype=np.float32)
    b2 = np.asarray(b2, dtype=np.float32)
    Ws = np.asarray(Ws, dtype=np.float32)
    bs = np.asarray(bs, dtype=np.float32)
    Wo = np.asarray(Wo, dtype=np.float32)
    bo = np.asarray(bo, dtype=np.float32)

    q = np.ascontiguousarray(x[:, 0, :], dtype=np.float32)       # [B, 12]
    force_prev = np.ascontiguousarray(x[:, 0, 6:9], dtype=np.float32)

    # Fold the two linear layers that have no nonlinearity between them.
    Wso = (Ws.astype(np.float64) @ Wo.astype(np.float64)).astype(np.float32)
    bso = (bs.astype(np.float64) @ Wo.astype(np.float64)
           + bo.astype(np.float64)).astype(np.float32)

    wpack = np.zeros((128, WCOLS), np.float32)
    for c in range(CHUNKS):
        wpack[c * 12:(c + 1) * 12, c * 16 + 0:(c + 1) * 16] = W1
        wpack[c * 16:(c + 1) * 16, 128 + c * 8:128 + (c + 1) * 8] = W2
        wpack[c * 8:(c + 1) * 8, 192 + c * 3:192 + (c + 1) * 3] = Wso
        wpack[c * 16:(c + 1) * 16, 216] = b1
        wpack[c * 8:(c + 1) * 8, 217] = b2

    in_maps = []
    for c in range(N_CORES):
        qc = q[c * BPC:(c + 1) * BPC]
        # atom n = t*4096 + ch*512 + a  ->  partition 12*ch+f, free t*512+a
        Ac = np.ascontiguousarray(
            qc.reshape(SUPER, CHUNKS, TILE_N, F)
              .transpose(1, 3, 0, 2).reshape(CHUNKS * F, FREE))
        in_maps.append({"xin": Ac, "wpack": wpack})
    return in_maps, force_prev, bso


def _host_gather(results, bso):
    out = np.empty((B, 3), np.float32)
    for c in range(N_CORES):
        Oc = results[c]["out"]                                   # [24, 4096]
        oc = (Oc.reshape(CHUNKS, 3, SUPER, TILE_N)
                .transpose(2, 0, 3, 1).reshape(BPC, 3))
        out[c * BPC:(c + 1) * BPC] = oc + bso
    return out


LAST_RESULT = None


def kernel(x, W1, b1, W2, b2, Ws, bs, Wo, bo):
    from concourse.bass_utils import run_bass_kernel_spmd

    in_maps, force_prev, bso = _host_prep(x, W1, b1, W2, b2, Ws, bs, Wo, bo)
    nc = _build_nc()
    res = run_bass_kernel_spmd(nc, in_maps, core_ids=list(range(N_CORES)))
    globals()["LAST_RESULT"] = res
    out = _host_gather(res.results, bso)
    return (out, force_prev)



# revision 7
# speedup vs baseline: 1.5269x; 1.5269x over previous
"""Trainium2 Bass kernel for nn_LIMADNN2_42013370090068 (dense_mlp).

Reference semantics: out depends only on x[:, 0, :] — the `state.add(...)`
neighbor loop in the torch module is not in-place, so the 65-neighbor
dimension is dead. force_prev = x[:, 0, 6:9] is a pure slice.

  q   = x[:, 0, :]                 # [B, 12]
  h   = relu(q @ W1 + b1)          # [B, 16]
  blk = relu(h @ W2 + b2)          # [B, 8]
  out = (blk @ Ws + bs) @ Wo + bo  # [B, 3]   (no relu between -> folded)

Device strategy (pure data parallel, 8 cores, batch-sharded, fp16):
  * Host slices q (12.6 MB of the 818 MB input) and casts to fp16; all
    matmul operands are fp16 (1 cyc/col on PE vs 4 for fp32).
  * Atoms stream along the matmul free dim in 512-col tiles; weights are
    block-diagonal and packed into 32x32 / 64x32 / 128x48 tile_position
    sub-arrays so concurrent PE tiles process 32/32/16 chunks per
    512-cycle stream (chunk = 512 atoms).
  * Per core: 64 chunks = 2 supertiles of 32 chunks. PSUM: L1 -> banks
    0-3, L2 -> 4-5, L3 -> 6/7 (double-buffered).
  * PSUM drains (the throughput floor: only ScalarE+VectorE reach PSUM,
    1 col/cycle fp32) are split between ScalarE ACTIVATE (bias+relu
    fused) and VectorE dual-op tensor_scalar, balanced by engine clock.
  * A dummy relu at t=0 prefetches the ACT spline table set (~1.3 us)
    under the input DMA.
"""

import numpy as np

B = 262144
F = 12
N_CORES = 8
BPC = B // N_CORES          # 32768 atoms per core
TN = 512                    # atoms per chunk == matmul free dim
NSUP = 2                    # supertiles per core, 32 chunks each

# ScalarE(1.2 GHz) / VectorE(0.96 GHz) drain split points (columns)
C1S = 1152                  # of 2048  (relu1)
C2S = 576                   # of 1024  (relu2)
C3S = 288                   # of 512   (out bias-add)

W1OFF = 0                   # wpack column offsets
W2OFF = 128
W3OFF = 256                 # block I=0 at 256, I=1 at 320


def _build_nc():
    import concourse.tile as tile
    from concourse import bacc, mybir

    f16 = mybir.dt.float16
    f32 = mybir.dt.float32

    nc = bacc.Bacc("TRN2", target_bir_lowering=False, debug=False,
                   num_devices=N_CORES)

    xin = nc.dram_tensor("xin", [96, 4096], f16, kind="ExternalInput")
    wpack = nc.dram_tensor("wpack", [128, 368], f16, kind="ExternalInput")
    bpack = nc.dram_tensor("bpack", [128, 4], f32, kind="ExternalInput")
    out = nc.dram_tensor("out", [112, 1024], f16, kind="ExternalOutput")

    Relu = mybir.ActivationFunctionType.Relu
    Ident = mybir.ActivationFunctionType.Identity
    add, vmax = mybir.AluOpType.add, mybir.AluOpType.max

    with tile.TileContext(nc) as tc:
        with (
            tc.tile_pool(name="const", bufs=1) as cpool,
            tc.tile_pool(name="x", bufs=1) as xpool,
            tc.tile_pool(name="h", bufs=1) as hpool,
            tc.tile_pool(name="blk", bufs=1) as bpool,
            tc.tile_pool(name="osb", bufs=1) as opool,
            tc.tile_pool(name="ps1", bufs=1, space="PSUM") as ps1pool,
            tc.tile_pool(name="ps2", bufs=1, space="PSUM") as ps2pool,
            tc.tile_pool(name="ps3", bufs=2, space="PSUM") as ps3pool,
        ):
            wsb = cpool.tile([128, 368], f16)
            bsb = cpool.tile([128, 4], f32)
            scr = cpool.tile([1, 2], f32)
            nc.sync.dma_start(wsb[:], wpack[:])
            nc.sync.dma_start(bsb[:], bpack[:])

            # Prefetch the ACT table set (relu+identity) under the DMAs.
            nc.vector.memset(scr[:], 0.0)
            nc.scalar.activation(scr[0:1, 1:2], scr[0:1, 0:1], Relu)
            nc.scalar.activation(scr[0:1, 1:2], scr[0:1, 0:1], Ident)

            xsb = xpool.tile([128, 4096], f16)
            for s in range(NSUP):
                for i in range(4):
                    nc.sync.dma_start(
                        xsb[32 * i:32 * i + 24, 2048 * s:2048 * s + 2048],
                        xin[24 * i:24 * i + 24, 2048 * s:2048 * s + 2048])

            hsb = hpool.tile([128, 4096], f16)
            blksb = bpool.tile([128, 2048], f16)
            osb = opool.tile([112, 1024], f16)

            for s in range(NSUP):
                xof = 2048 * s

                ps1 = ps1pool.tile([128, 2048], f32)
                for i in range(4):
                    for j in range(4):
                        nc.tensor.matmul(
                            ps1[32 * j:32 * j + 32, TN * i:TN * i + TN],
                            wsb[32 * i:32 * i + 24,
                                W1OFF + 32 * j:W1OFF + 32 * j + 32],
                            xsb[32 * i:32 * i + 24,
                                xof + TN * j:xof + TN * j + TN],
                            start=True, stop=True,
                            tile_position=(32 * i, 32 * j))

                hof = 2048 * s
                nc.scalar.activation(hsb[:, hof:hof + C1S], ps1[:, 0:C1S],
                                     Relu, bias=bsb[:, 0:1])
                nc.vector.tensor_scalar(hsb[:, hof + C1S:hof + 2048],
                                        ps1[:, C1S:2048],
                                        bsb[:, 0:1], 0.0, add, vmax)

                ps2 = ps2pool.tile([128, 1024], f32)
                for half in range(2):
                    for i in range(4):
                        nc.tensor.matmul(
                            ps2[32 * i:32 * i + 32, TN * half:TN * half + TN],
                            wsb[64 * half:64 * half + 64,
                                W2OFF + 32 * i:W2OFF + 32 * i + 32],
                            hsb[64 * half:64 * half + 64,
                                hof + TN * i:hof + TN * i + TN],
                            start=True, stop=True,
                            tile_position=(64 * half, 32 * i))

                bof = 1024 * s
                nc.scalar.activation(blksb[:, bof:bof + C2S], ps2[:, 0:C2S],
                                     Relu, bias=bsb[:, 1:2])
                nc.vector.tensor_scalar(blksb[:, bof + C2S:bof + 1024],
                                        ps2[:, C2S:1024],
                                        bsb[:, 1:2], 0.0, add, vmax)

                ps3 = ps3pool.tile([128, TN], f32)
                nc.tensor.matmul(ps3[0:48, :],
                                 wsb[0:128, W3OFF:W3OFF + 48],
                                 blksb[:, bof:bof + TN],
                                 start=True, stop=True, tile_position=(0, 0))
                nc.tensor.matmul(ps3[64:112, :],
                                 wsb[0:128, W3OFF + 64:W3OFF + 112],
                                 blksb[:, bof + TN:bof + 2 * TN],
                                 start=True, stop=True, tile_position=(0, 64))

                oof = TN * s
                nc.scalar.activation(osb[0:112, oof:oof + C3S],
                                     ps3[0:112, 0:C3S],
                                     Ident, bias=bsb[0:112, 2:3])
                nc.vector.tensor_scalar(osb[0:112, oof + C3S:oof + TN],
                                        ps3[0:112, C3S:TN],
                                        bsb[0:112, 2:3], None, add)
                nc.sync.dma_start(out[0:112, oof:oof + TN],
                                  osb[0:112, oof:oof + TN])

    nc.finalize()
    return nc


def _host_prep(x, W1, b1, W2, b2, Ws, bs, Wo, bo):
    x = np.asarray(x)
    W1 = np.asarray(W1, dtype=np.float32)
    b1 = np.asarray(b1, dtype=np.float32)
    W2 = np.asarray(W2, dtype=np.float32)
    b2 = np.asarray(b2, dtype=np.float32)
    Ws = np.asarray(Ws, dtype=np.float32)
    bs = np.asarray(bs, dtype=np.float32)
    Wo = np.asarray(Wo, dtype=np.float32)
    bo = np.asarray(bo, dtype=np.float32)

    q = np.ascontiguousarray(x[:, 0, :], dtype=np.float32)       # [B, 12]
    force_prev = np.ascontiguousarray(x[:, 0, 6:9], dtype=np.float32)

    # Fold the two linear layers that have no nonlinearity between them.
    Wso = (Ws.astype(np.float64) @ Wo.astype(np.float64)).astype(np.float32)
    bso = (bs.astype(np.float64) @ Wo.astype(np.float64)
           + bo.astype(np.float64)).astype(np.float32)

    W1h = W1.astype(np.float16)
    W2h = W2.astype(np.float16)
    W3h = Wso.astype(np.float16)

    wpack = np.zeros((128, 368), np.float16)
    w1b = np.zeros((24, 32), np.float16)            # 2-chunk block-diag
    w1b[0:12, 0:16] = W1h
    w1b[12:24, 16:32] = W1h
    for i in range(4):
        for j in range(4):
            wpack[32 * i:32 * i + 24, W1OFF + 32 * j:W1OFF + 32 * j + 32] = w1b
    w2b = np.zeros((64, 32), np.float16)            # 4-chunk block-diag
    for m in range(4):
        w2b[16 * m:16 * m + 16, 8 * m:8 * m + 8] = W2h
    for half in range(2):
        for g in range(4):
            wpack[64 * half:64 * half + 64,
                  W2OFF + 32 * g:W2OFF + 32 * g + 32] = w2b
    w3b = np.zeros((128, 48), np.float16)           # 16-chunk block-diag
    for t in range(16):
        w3b[8 * t:8 * t + 8, 3 * t:3 * t + 3] = W3h
    wpack[:, W3OFF:W3OFF + 48] = w3b
    wpack[:, W3OFF + 64:W3OFF + 112] = w3b

    bpack = np.zeros((128, 4), np.float32)
    bpack[:, 0] = np.tile(b1, 8)
    bpack[:, 1] = np.tile(b2, 16)
    bpack[0:48, 2] = np.tile(bso, 16)
    bpack[64:112, 2] = np.tile(bso, 16)

    in_maps = []
    for c in range(N_CORES):
        qc = q[c * BPC:(c + 1) * BPC].astype(np.float16)
        # chunk c = 32s + 8i + 2j + k of 512 atoms; DRAM layout is
        # [row = 24i + 12k + f, col = 2048s + 512j + a].
        t = qc.reshape(2, 4, 4, 2, TN, F)           # s i j k a f
        xc = np.ascontiguousarray(
            t.transpose(1, 3, 5, 0, 2, 4).reshape(96, 4096))
        in_maps.append({"xin": xc, "wpack": wpack, "bpack": bpack})
    return in_maps, force_prev


def _host_gather(results):
    out = np.empty((B, 3), np.float32)
    tt = np.arange(16)
    for c in range(N_CORES):
        Oc = results[c]["out"]                      # [112, 1024] fp16
        oc = np.empty((64, TN, 3), np.float32)
        for half in range(2):
            blkO = Oc[64 * half:64 * half + 48].astype(np.float32)
            blkO = blkO.reshape(16, 3, 2, TN)       # t r s a
            cc = 8 * (tt // 4) + 4 * half + (tt % 4)
            for s in range(2):
                oc[32 * s + cc] = blkO[:, :, s, :].transpose(0, 2, 1)
        out[c * BPC:(c + 1) * BPC] = oc.reshape(BPC, 3)
    return out


LAST_RESULT = None


def kernel(x, W1, b1, W2, b2, Ws, bs, Wo, bo):
    from concourse.bass_utils import run_bass_kernel_spmd

    in_maps, force_prev = _host_prep(x, W1, b1, W2, b2, Ws, bs, Wo, bo)
    nc = _build_nc()
    res = run_bass_kernel_spmd(nc, in_maps, core_ids=list(range(N_CORES)))
    globals()["LAST_RESULT"] = res
    out = _host_gather(res.results)
    return (out, force_prev)


# revision 19
# speedup vs baseline: 2.0567x; 1.3470x over previous
"""Trainium2 Bass kernel for nn_LIMADNN2_42013370090068 (dense_mlp).

Reference semantics: out depends only on x[:, 0, :] — the `state.add(...)`
neighbor loop in the torch module is not in-place, so the 65-neighbor
dimension is dead. force_prev = x[:, 0, 6:9] is a pure slice.

  q   = x[:, 0, :]                 # [B, 12]
  h   = relu(q @ W1 + b1)          # [B, 16]
  blk = relu(h @ W2 + b2)          # [B, 8]
  out = (blk @ Ws + bs) @ Wo + bo  # [B, 3]   (no relu between -> folded)

Device strategy (pure data parallel, 8 cores, batch-sharded, fp16):
  * Host slices q (12.6 MB of the 818 MB input), casts fp16; all matmul
    operands fp16 (1 cyc/col on PE vs 4 for fp32).
  * Full-width block-diagonal matmuls stream atoms on the free dim at
    the per-layer packing limit: L1 [96x128] = 8 chunks/stream,
    L2 [128x64] col-paired = 8/stream, L3 [128x48] col-paired =
    16/stream -> 20 matmuls/core, 10240 streamed columns total (the PE
    floor for this shape). Back-to-back emission keeps the PE busy so
    the HAM clock-gate ramps to 2.4 GHz mid-kernel.
  * One 8-bank PSUM tile, manually sliced: L1 -> banks 0-7, L2 reuses
    0-3, L3 reuses 4-5 (range-tracked WAR via drains).
  * PSUM drains (only ScalarE+VectorE reach PSUM, 1 col/cyc fp32) are
    split across both engines, bias+relu fused.
  * A dummy relu at t=0 prefetches the ACT spline table (~1.3 us) under
    the fixed preamble.
"""

import numpy as np

B = 262144
F = 12
N_CORES = 8
BPC = B // N_CORES          # 32768 atoms per core
TN = 512                    # atoms per chunk == matmul free dim == psum bank
NP = 8                      # L1 passes (8 chunks of 8 atoms-chunks each)

W1OFF = 0                   # wpack column offsets
W2OFF = 128
W3OFF = 192                 # block a at 192, block b at 256


def _build_nc():
    import concourse.tile as tile
    from concourse import bacc, mybir

    f16 = mybir.dt.float16
    f32 = mybir.dt.float32

    nc = bacc.Bacc("TRN2", target_bir_lowering=False, debug=False,
                   num_devices=N_CORES)

    xin = nc.dram_tensor("xin", [96, 4096], f16, kind="ExternalInput")
    wpack = nc.dram_tensor("wpack", [128, 312], f16, kind="ExternalInput")
    out = nc.dram_tensor("out", [112, 1024], f16, kind="ExternalOutput")

    Relu = mybir.ActivationFunctionType.Relu
    Ident = mybir.ActivationFunctionType.Identity
    add, vmax = mybir.AluOpType.add, mybir.AluOpType.max

    with tile.TileContext(nc) as tc:
        with (
            tc.tile_pool(name="const", bufs=1) as cpool,
            tc.tile_pool(name="x", bufs=1) as xpool,
            tc.tile_pool(name="h", bufs=1) as hpool,
            tc.tile_pool(name="blk", bufs=1) as bpool,
            tc.tile_pool(name="osb", bufs=1) as opool,
            tc.tile_pool(name="ps", bufs=1, space="PSUM") as pspool,
        ):
            wsb = cpool.tile([128, 312], f16)
            scr = cpool.tile([1, 2], f32)
            nc.scalar.dma_start(wsb[:], wpack[:])
            bview = wsb[0:128, 304:312].bitcast(f32)  # fp32 biases, packed
            b1a = bview[0:128, 0:1]
            b2a = bview[0:128, 1:2]
            boa = bview[0:112, 2:3]

            # Prefetch the ACT table set (relu+identity) under the preamble.
            nc.vector.memset(scr[:], 0.0)
            nc.scalar.activation(scr[0:1, 1:2], scr[0:1, 0:1], Relu)

            xsb = xpool.tile([96, 4096], f16)
            for b in range(4):
                nc.sync.dma_start(xsb[:, 1024 * b:1024 * b + 1024],
                                  xin[:, 1024 * b:1024 * b + 1024])

            hsb = hpool.tile([128, 4096], f16)
            blksb = bpool.tile([128, 2048], f16)
            osb = opool.tile([112, 1024], f16)
            ps = pspool.tile([128, 4096], f32)      # all 8 banks

            # L1: 8 full-width matmuls, 8 chunks each -> banks 0-7.
            for p in range(NP):
                nc.tensor.matmul(ps[0:128, TN * p:TN * p + TN],
                                 wsb[0:96, W1OFF:W1OFF + 128],
                                 xsb[0:96, TN * p:TN * p + TN],
                                 start=True, stop=True)

            # relu1 drains: 2-bank ops, alternating Scalar/Vector.
            for d in range(4):
                lo = 1024 * d
                if d % 2 == 0:
                    nc.scalar.activation(hsb[:, lo:lo + 1024],
                                         ps[:, lo:lo + 1024],
                                         Relu, bias=b1a)
                else:
                    nc.vector.tensor_scalar(hsb[:, lo:lo + 1024],
                                            ps[:, lo:lo + 1024],
                                            b1a, 0.0, add, vmax)

            # L2: 4 passes x 2 col-paired matmuls -> banks 0-3 (reused).
            for q in range(4):
                for e in range(2):
                    nc.tensor.matmul(
                        ps[64 * e:64 * e + 64, TN * q:TN * q + TN],
                        wsb[0:128, W2OFF:W2OFF + 64],
                        hsb[0:128, TN * (2 * q + e):TN * (2 * q + e) + TN],
                        start=True, stop=True)

            # relu2 drains: 2-bank ops.
            for d in range(2):
                lo = 1024 * d
                if d % 2 == 0:
                    nc.scalar.activation(blksb[:, lo:lo + 1024],
                                         ps[:, lo:lo + 1024],
                                         Relu, bias=b2a)
                else:
                    nc.vector.tensor_scalar(blksb[:, lo:lo + 1024],
                                            ps[:, lo:lo + 1024],
                                            b2a, 0.0, add, vmax)

            # L3: 2 passes x 2 col-paired matmuls -> banks 4-5 (reused).
            for r in range(2):
                for m in range(2):
                    nc.tensor.matmul(
                        ps[64 * m:64 * m + 48,
                           TN * (4 + r):TN * (4 + r) + TN],
                        wsb[0:128, W3OFF + 64 * m:W3OFF + 64 * m + 48],
                        blksb[0:128, TN * (2 * r + m):TN * (2 * r + m) + TN],
                        start=True, stop=True)

            # out drains (+bso): Scalar one bank, Vector the other.
            nc.scalar.activation(osb[0:112, 0:512],
                                 ps[0:112, TN * 4:TN * 4 + 512],
                                 Ident, bias=boa)
            nc.vector.tensor_scalar(osb[0:112, 512:1024],
                                    ps[0:112, TN * 5:TN * 5 + 512],
                                    boa, None, add)
            nc.sync.dma_start(out[0:112, 0:512], osb[0:112, 0:512])
            nc.sync.dma_start(out[0:112, 512:1024], osb[0:112, 512:1024])

    nc.finalize()
    return nc


def _host_prep(x, W1, b1, W2, b2, Ws, bs, Wo, bo):
    x = np.asarray(x)
    W1 = np.asarray(W1, dtype=np.float32)
    b1 = np.asarray(b1, dtype=np.float32)
    W2 = np.asarray(W2, dtype=np.float32)
    b2 = np.asarray(b2, dtype=np.float32)
    Ws = np.asarray(Ws, dtype=np.float32)
    bs = np.asarray(bs, dtype=np.float32)
    Wo = np.asarray(Wo, dtype=np.float32)
    bo = np.asarray(bo, dtype=np.float32)

    q = np.ascontiguousarray(x[:, 0, :], dtype=np.float32)       # [B, 12]
    force_prev = np.ascontiguousarray(x[:, 0, 6:9], dtype=np.float32)

    # Fold the two linear layers that have no nonlinearity between them.
    Wso = (Ws.astype(np.float64) @ Wo.astype(np.float64)).astype(np.float32)
    bso = (bs.astype(np.float64) @ Wo.astype(np.float64)
           + bo.astype(np.float64)).astype(np.float32)

    W1h = W1.astype(np.float16)
    W2h = W2.astype(np.float16)
    W3h = Wso.astype(np.float16)

    wpack = np.zeros((128, 312), np.float16)
    for c8 in range(8):                             # L1: 8-chunk block-diag
        wpack[12 * c8:12 * c8 + 12,
              W1OFF + 16 * c8:W1OFF + 16 * c8 + 16] = W1h
    for c8 in range(8):                             # L2: 8-chunk block-diag
        wpack[16 * c8:16 * c8 + 16,
              W2OFF + 8 * c8:W2OFF + 8 * c8 + 8] = W2h
    w3b = np.zeros((128, 48), np.float16)           # L3: 16-chunk block-diag
    for e in range(2):
        for c8 in range(8):
            t = 8 * e + c8
            w3b[64 * e + 8 * c8:64 * e + 8 * c8 + 8,
                3 * t:3 * t + 3] = W3h
    wpack[:, W3OFF:W3OFF + 48] = w3b
    wpack[:, W3OFF + 64:W3OFF + 112] = w3b

    bias32 = np.zeros((128, 4), np.float32)
    bias32[:, 0] = np.tile(b1, 8)
    bias32[:, 1] = np.tile(b2, 16)
    bias32[0:48, 2] = np.tile(bso, 16)
    bias32[64:112, 2] = np.tile(bso, 16)
    wpack[:, 304:312] = bias32.view(np.float16)

    in_maps = []
    for c in range(N_CORES):
        qc = q[c * BPC:(c + 1) * BPC].astype(np.float16)
        # chunk c = 8p + c8 (512 atoms); [row = 12*c8 + f, col = 512p + a]
        t = qc.reshape(NP, 8, TN, F)                # p c8 a f
        xc = np.ascontiguousarray(
            t.transpose(1, 3, 0, 2).reshape(96, 4096))
        in_maps.append({"xin": xc, "wpack": wpack})
    return in_maps, force_prev


def _host_gather(results):
    out = np.empty((B, 3), np.float32)
    for c in range(N_CORES):
        Oc = results[c]["out"]                      # [112, 1024] fp16
        oc = np.empty((64, TN, 3), np.float32)
        for r in range(2):
            for m in range(2):
                blkO = Oc[64 * m:64 * m + 48,
                          TN * r:TN * r + TN].astype(np.float32)
                # row = 3t + rr, t = 8e + c8; chunk = 32r + 16m + t
                oc[32 * r + 16 * m:32 * r + 16 * m + 16] = (
                    blkO.reshape(16, 3, TN).transpose(0, 2, 1))
        out[c * BPC:(c + 1) * BPC] = oc.reshape(BPC, 3)
    return out


LAST_RESULT = None


def kernel(x, W1, b1, W2, b2, Ws, bs, Wo, bo):
    from concourse.bass_utils import run_bass_kernel_spmd

    in_maps, force_prev = _host_prep(x, W1, b1, W2, b2, Ws, bs, Wo, bo)
    nc = _build_nc()
    res = run_bass_kernel_spmd(nc, in_maps, core_ids=list(range(N_CORES)))
    globals()["LAST_RESULT"] = res
    out = _host_gather(res.results)
    return (out, force_prev)
